# revision 1
# baseline (speedup 1.0000x reference)
"""Multi-head self-attention on 8 Trainium2 NeuronCores.

Problem: X[2,2048,2048] -> MHA(16 heads, head_dim 128) -> [2,2048,2048].

Sharding: core c in 0..7 handles batch b = c // 4 and head-group g = c % 4
(4 heads = 512 hidden columns per core).  Each core computes
    Q^T,K^T,V  (its 512-column slice of the QKV projections)
    per-head attention (softmax without max-subtraction; inputs are bounded)
    partial output projection  out_heads @ Wo[512-slice, :]  -> [2048, 2048]
The 4 partial projections per batch are summed on the host (the tensor-
parallel "all-reduce" is done in numpy) and the output bias is added there.

Per-core dataflow (all matmul operands land in their natural layouts; no
on-device transposes anywhere):
    XT = X[b].T                          (host-prepared, bf16, [H, S])
    Q^T[512,S] = matmul(lhsT=Wq_slice, rhs=XT)       (Wq pre-scaled 1/sqrt(d))
    K^T[512,S] = matmul(lhsT=Wk_slice, rhs=XT)
    V  [S,512] = matmul(lhsT=XT,       rhs=Wv_slice)
    scoresT[k,q] = matmul(lhsT=K^T_h tile, rhs=Q^T_h chunk)   (d=128 contract)
    expT = Exp(scoresT)  (ACT, bf16 out)
    denom: DVE pairwise-add tree over the 16 k-tiles, then a single
           ones[128,128] @ partial  matmul -> broadcast row-sums, reciprocal
    outT_h[d,q] = matmul(lhsT=V_h tiles, rhs=expT tiles)  * recip(denom)
    y[q,:] += matmul(lhsT=outT tiles, rhs=Wo_slice)       (partial, f32 out)
"""

import math
import os
import sys

import numpy as np

sys.path.insert(0, "/opt/trn_rl_repo")

import ml_dtypes  # noqa: E402

import concourse.bass as bass  # noqa: E402
import concourse.bass_isa as bass_isa  # noqa: E402
import concourse.mybir as mybir  # noqa: E402
import concourse.tile as tile  # noqa: E402
from concourse import bacc  # noqa: E402

B, S, H = 2, 2048, 2048
HEADS, D = 16, 128
NC = 8
GROUPS = 4            # cores per batch (head-group parallel)
CW = H // GROUPS      # 512 hidden columns per core (4 heads)
HG = CW // D          # 4 heads per core
P = 128               # partitions
FN = 512              # matmul free-dim / psum bank (f32)
KT = H // P           # 16 contraction tiles for the projections
SQ = S // FN          # 4 query chunks of 512
SK = S // P           # 16 key tiles of 128

BF16 = mybir.dt.bfloat16
F32 = mybir.dt.float32

_CACHE = {}


def _build_nc():
    nc = bacc.Bacc()
    xt = nc.dram_tensor("xt", [H, S], BF16, kind="ExternalInput")
    wq = nc.dram_tensor("wq", [H, CW], BF16, kind="ExternalInput")
    wk = nc.dram_tensor("wk", [H, CW], BF16, kind="ExternalInput")
    wv = nc.dram_tensor("wv", [H, CW], BF16, kind="ExternalInput")
    wo = nc.dram_tensor("wo", [CW, H], BF16, kind="ExternalInput")
    bq = nc.dram_tensor("bq", [CW], F32, kind="ExternalInput")
    bk = nc.dram_tensor("bk", [CW], F32, kind="ExternalInput")
    bv = nc.dram_tensor("bv", [CW], F32, kind="ExternalInput")
    y = nc.dram_tensor("y", [S, H], F32, kind="ExternalOutput")

    with tile.TileContext(nc) as tc:
        _emit(nc, tc, xt[:], wq[:], wk[:], wv[:], wo[:], bq[:], bk[:], bv[:], y[:])
    nc.finalize()
    return nc


def _emit(nc, tc, xt, wq, wk, wv, wo, bq, bk, bv, y):
    from contextlib import ExitStack

    with ExitStack() as ctx:
        consts = ctx.enter_context(tc.tile_pool(name="consts", bufs=1))
        qkv = ctx.enter_context(tc.tile_pool(name="qkv", bufs=1))
        psum_proj = ctx.enter_context(tc.tile_pool(name="psum_proj", bufs=2, space="PSUM"))
        psum_s = ctx.enter_context(tc.tile_pool(name="psum_s", bufs=2, space="PSUM"))
        psum_av = ctx.enter_context(tc.tile_pool(name="psum_av", bufs=2, space="PSUM"))

        # --- constants (gpsimd queue: keep them off the bulk-load path) ----
        bq_s = consts.tile([P, HG], F32)
        bk_s = consts.tile([P, HG], F32)
        with nc.allow_non_contiguous_dma(reason="tiny one-time bias load"):
            nc.gpsimd.dma_start(bq_s, bq.rearrange("(m p) -> p m", p=P))
            nc.gpsimd.dma_start(bk_s, bk.rearrange("(m p) -> p m", p=P))
        bv_row = consts.tile([P, CW], F32)
        nc.gpsimd.dma_start(bv_row, bv[None, :].to_broadcast([P, CW]))
        ones_s = consts.tile([P, P], BF16)
        nc.vector.memset(ones_s, 1.0)

        qt = qkv.tile([P, HG, S], BF16)   # Q^T: [d-part, head, S]
        kt_sb = qkv.tile([P, HG, S], BF16)
        v_sb = qkv.tile([P, SK, CW], BF16)  # V: [S-part(tile), S-tile, 4 heads*d]

        # --- phase A: projections -----------------------------------------
        with tc.tile_pool(name="xtw", bufs=1) as xtw:
            # split the big input loads per contraction tile so the first
            # projection matmuls can start as soon as chunk 0 lands
            xts = xtw.tile([P, KT, S], BF16)
            wq_s = xtw.tile([P, KT, CW], BF16)
            wk_s = xtw.tile([P, KT, CW], BF16)
            wv_s = xtw.tile([P, KT, CW], BF16)
            xt_r = xt.rearrange("(a p) s -> p a s", p=P)
            wq_r = wq.rearrange("(a p) m -> p a m", p=P)
            wk_r = wk.rearrange("(a p) m -> p a m", p=P)
            wv_r = wv.rearrange("(a p) m -> p a m", p=P)
            for k in range(KT):
                nc.sync.dma_start(wk_s[:, k], wk_r[:, k])
                nc.sync.dma_start(xts[:, k], xt_r[:, k])
            for k in range(KT):
                nc.sync.dma_start(wq_s[:, k], wq_r[:, k])
                nc.sync.dma_start(wv_s[:, k], wv_r[:, k])

            for w_s, b_s, dst in ((wk_s, bk_s, kt_sb), (wq_s, bq_s, qt)):
                for m in range(HG):
                    for c in range(SQ):
                        ps = psum_proj.tile([P, FN], F32, tag="ps", name="ps")
                        for k in range(KT):
                            nc.tensor.matmul(
                                ps,
                                w_s[:, k, m * P:(m + 1) * P],
                                xts[:, k, c * FN:(c + 1) * FN],
                                start=(k == 0),
                                stop=(k == KT - 1),
                            )
                        nc.vector.tensor_scalar_add(
                            dst[:, m, c * FN:(c + 1) * FN], ps, b_s[:, m:m + 1])

            for m in range(SK):
                ps = psum_proj.tile([P, FN], F32)
                for k in range(KT):
                    nc.tensor.matmul(
                        ps,
                        xts[:, k, m * P:(m + 1) * P],
                        wv_s[:, k, :],
                        start=(k == 0),
                        stop=(k == KT - 1),
                    )
                nc.vector.tensor_add(v_sb[:, m, :], ps, bv_row)

        # --- phase B+C pools ----------------------------------------------
        wo_pool = ctx.enter_context(tc.tile_pool(name="wo", bufs=1))
        wo_s = wo_pool.tile([P, HG, H], BF16)
        nc.sync.dma_start(wo_s, wo.rearrange("(a p) n -> p a n", p=P))

        outt = ctx.enter_context(tc.tile_pool(name="outt", bufs=1)).tile(
            [P, HG, S], BF16)  # out_heads^T: [d-part, head, S]

        exp_pool = ctx.enter_context(tc.tile_pool(name="expt", bufs=4))
        red_pool = ctx.enter_context(tc.tile_pool(name="red", bufs=2))
        rden_pool = ctx.enter_context(tc.tile_pool(name="rden", bufs=2))
        ystage = ctx.enter_context(tc.tile_pool(name="ystage", bufs=3))

        # --- phases B+C interleaved per q-chunk ----------------------------
        # B unit (h, c): scoresT -> exp (2-ktile-wide ACT ops) -> denom -> AV
        # After all 4 heads of chunk c, project chunk c's rows (phase C).
        for c in range(SQ):
            for h in range(HG):
                et = exp_pool.tile([P, SK, FN], BF16)
                for j in range(SK // 2):
                    ps_s = psum_s.tile([P, 2, FN], F32)
                    for i in range(2):
                        nc.tensor.matmul(
                            ps_s[:, i],
                            kt_sb[:, h, (2 * j + i) * P:(2 * j + i + 1) * P],
                            qt[:, h, c * FN:(c + 1) * FN],
                            start=True, stop=True,
                        )
                    nc.scalar.activation(
                        et[:, 2 * j:2 * j + 2, :].rearrange("p a q -> p (a q)"),
                        ps_s.rearrange("p a q -> p (a q)"),
                        mybir.ActivationFunctionType.Exp)

                ps_d = psum_proj.tile([P, FN], F32, tag="ps", name="ps_d")
                if c < SQ - 1:
                    # DVE pairwise-add tree over the 16 k-tiles, then one
                    # ones-matmul to broadcast the row sums across partitions
                    s8 = red_pool.tile([P, 8, FN], BF16, tag="s8")
                    nc.vector.tensor_add(s8, et[:, 0:8, :], et[:, 8:16, :])
                    s4 = red_pool.tile([P, 4, FN], BF16, tag="s4")
                    nc.vector.tensor_add(s4, s8[:, 0:4, :], s8[:, 4:8, :])
                    s2 = red_pool.tile([P, 2, FN], BF16, tag="s2")
                    nc.vector.tensor_add(s2, s4[:, 0:2, :], s4[:, 2:4, :])
                    dsum = red_pool.tile([P, FN], BF16, tag="dsum")
                    nc.vector.tensor_add(dsum, s2[:, 0, :], s2[:, 1, :])
                    nc.tensor.matmul(ps_d, ones_s, dsum, start=True, stop=True)
                else:
                    # final chunk: the DVE tree would sit on the drain's
                    # critical path while PE idles — accumulate on PE instead
                    for k in range(SK):
                        nc.tensor.matmul(ps_d, ones_s, et[:, k, :],
                                         start=(k == 0), stop=(k == SK - 1))
                rden = rden_pool.tile([P, FN], F32)
                nc.vector.reciprocal_approx_fast(out=rden, in_=ps_d)

                ps_o = psum_av.tile([P, FN], F32)
                for k in range(SK):
                    nc.tensor.matmul(
                        ps_o,
                        v_sb[:, k, h * P:(h + 1) * P],
                        et[:, k, :],
                        start=(k == 0),
                        stop=(k == SK - 1),
                    )
                nc.vector.tensor_mul(
                    outt[:, h, c * FN:(c + 1) * FN], ps_o, rden)

            # phase C for the S-rows covered by chunk c
            for m in range(4 * c, 4 * (c + 1)):
                for c2 in range(H // FN):
                    ps = psum_proj.tile([P, FN], F32)
                    for k in range(HG):
                        nc.tensor.matmul(
                            ps,
                            outt[:, k, m * P:(m + 1) * P],
                            wo_s[:, k, c2 * FN:(c2 + 1) * FN],
                            start=(k == 0),
                            stop=(k == HG - 1),
                        )
                    yt = ystage.tile([P, FN], F32)
                    nc.vector.tensor_copy(yt, ps)
                    nc.sync.dma_start(
                        y[m * P:(m + 1) * P, c2 * FN:(c2 + 1) * FN], yt)


def _get_nc():
    if "nc" not in _CACHE:
        _CACHE["nc"] = _build_nc()
    return _CACHE["nc"]


def make_in_maps(X, Wq, bq, Wk, bk, Wv, bv, Wo, bo):
    bf16 = ml_dtypes.bfloat16
    scale = 1.0 / math.sqrt(D)
    X = np.asarray(X, dtype=np.float32)
    xt_b = [np.ascontiguousarray(X[b].T).astype(bf16) for b in range(B)]
    Wq = np.asarray(Wq, dtype=np.float32) * scale
    Wk = np.asarray(Wk, dtype=np.float32)
    Wv = np.asarray(Wv, dtype=np.float32)
    Wo = np.asarray(Wo, dtype=np.float32)
    bq = np.asarray(bq, dtype=np.float32) * scale
    bk = np.asarray(bk, dtype=np.float32)
    bv = np.asarray(bv, dtype=np.float32)
    in_maps = []
    for c in range(NC):
        b, g = divmod(c, GROUPS)
        sl = slice(g * CW, (g + 1) * CW)
        in_maps.append({
            "xt": xt_b[b],
            "wq": np.ascontiguousarray(Wq[:, sl]).astype(bf16),
            "wk": np.ascontiguousarray(Wk[:, sl]).astype(bf16),
            "wv": np.ascontiguousarray(Wv[:, sl]).astype(bf16),
            "wo": np.ascontiguousarray(Wo[sl, :]).astype(bf16),
            "bq": np.ascontiguousarray(bq[sl]),
            "bk": np.ascontiguousarray(bk[sl]),
            "bv": np.ascontiguousarray(bv[sl]),
        })
    return in_maps


def gather_output(results, bo):
    bo = np.asarray(bo, dtype=np.float32)
    out = np.empty((B, S, H), np.float32)
    for b in range(B):
        acc = results[b * GROUPS]["y"].astype(np.float32, copy=True)
        for g in range(1, GROUPS):
            acc += results[b * GROUPS + g]["y"]
        out[b] = acc + bo[None, :]
    return out


def kernel(X, Wq, bq, Wk, bk, Wv, bv, Wo, bo):
    from concourse.bass_utils import run_bass_kernel_spmd

    in_maps = make_in_maps(X, Wq, bq, Wk, bk, Wv, bv, Wo, bo)
    nc = _get_nc()
    res = run_bass_kernel_spmd(nc, in_maps, list(range(NC))).results
    return gather_output(res, bo)



# revision 11
# speedup vs baseline: 1.1930x; 1.1930x over previous
"""Multi-head self-attention on 8 Trainium2 NeuronCores.

Problem: X[2,2048,2048] -> MHA(16 heads, head_dim 128) -> [2,2048,2048].

Sharding: core c in 0..7 handles batch b = c // 4 and head-group g = c % 4
(4 heads = 512 hidden columns per core).  Each core computes
    Q^T,K^T,V  (its 512-column slice of the QKV projections)
    per-head attention (softmax without max-subtraction; inputs are bounded)
    partial output projection  out_heads @ Wo[512-slice, :]  -> [2048, 2048]
The 4 partial projections per batch are summed on the host (the tensor-
parallel "all-reduce" is done in numpy) and the output bias is added there.

The PE is pure column-rate-bound (~0.49 ns/col sustained; LDWEIGHTS and
per-instruction overhead fully hidden), so the schedule is built to keep
the PE column stream dense from t~0:

  - ~10 warm-up matmuls on memset data run while the first DMA chunks land,
    so the HAM clock-gate reaches 8/8 before real work starts.
  - Phase A starts with a "chase" pass: 8 K^T groups (heads 0,1 x all four
    S-chunks) accumulate in 8 PSUM banks with the contraction (k) loop
    OUTERMOST, so every arriving (wk,xt) chunk immediately feeds 8 matmuls.
    PE never has to wait for the full 10.5 MB input load.
  - Softmax denominators: an incremental DVE pair-add chase over the exp
    tiles followed by ONE ones-matmul per unit, emitted AFTER the AV
    accumulation (the tree never sits on the PE critical path).
  - Phase C (output projection) tiles for chunk c-1 are interleaved into
    chunk c's unit stream as PE filler while ACT catches up on exp.
"""

import math
import sys
from contextlib import ExitStack

import numpy as np

sys.path.insert(0, "/opt/trn_rl_repo")

import ml_dtypes  # noqa: E402

import concourse.bass as bass  # noqa: E402
import concourse.mybir as mybir  # noqa: E402
import concourse.tile as tile  # noqa: E402
from concourse import bacc  # noqa: E402

B, S, H = 2, 2048, 2048
HEADS, D = 16, 128
NC = 8
GROUPS = 4            # cores per batch (head-group parallel)
CW = H // GROUPS      # 512 hidden columns per core (4 heads)
HG = CW // D          # 4 heads per core
P = 128               # partitions
FN = 512              # matmul free-dim / psum bank (f32)
KT = H // P           # 16 contraction tiles for the projections
SQ = S // FN          # 4 query chunks of 512
SK = S // P           # 16 key tiles of 128
NWARM = 10            # HAM warm-up filler matmuls

BF16 = mybir.dt.bfloat16
F32 = mybir.dt.float32

_CACHE = {}


def _build_nc():
    nc = bacc.Bacc()
    xt = nc.dram_tensor("xt", [H, S], BF16, kind="ExternalInput")
    wq = nc.dram_tensor("wq", [H, CW], BF16, kind="ExternalInput")
    wk = nc.dram_tensor("wk", [H, CW], BF16, kind="ExternalInput")
    wv = nc.dram_tensor("wv", [H, CW], BF16, kind="ExternalInput")
    wo = nc.dram_tensor("wo", [CW, H], BF16, kind="ExternalInput")
    bq = nc.dram_tensor("bq", [CW], F32, kind="ExternalInput")
    bk = nc.dram_tensor("bk", [CW], F32, kind="ExternalInput")
    bv = nc.dram_tensor("bv", [CW], F32, kind="ExternalInput")
    y = nc.dram_tensor("y", [S, H], F32, kind="ExternalOutput")

    with tile.TileContext(nc) as tc:
        _emit(nc, tc, xt[:], wq[:], wk[:], wv[:], wo[:], bq[:], bk[:], bv[:], y[:])
    nc.finalize()
    return nc


def _emit(nc, tc, xt, wq, wk, wv, wo, bq, bk, bv, y):
    with ExitStack() as ctx:
        consts = ctx.enter_context(tc.tile_pool(name="consts", bufs=1))
        qkv = ctx.enter_context(tc.tile_pool(name="qkv", bufs=1))
        wo_pool = ctx.enter_context(tc.tile_pool(name="wo", bufs=1))
        # one shared PSUM pool: every tile is [P, 2, FN] f32 = 2 banks,
        # 4 rotating buffers = all 8 banks
        psum = ctx.enter_context(tc.tile_pool(name="psum", bufs=4, space="PSUM"))

        # --- constants ------------------------------------------------------
        ones_s = consts.tile([P, P], BF16)
        nc.vector.memset(ones_s, 1.0)
        warm_rhs = consts.tile([P, FN], BF16)
        nc.vector.memset(warm_rhs, 0.001)
        bq_s = consts.tile([P, HG], F32)
        bk_s = consts.tile([P, HG], F32)
        with nc.allow_non_contiguous_dma(reason="tiny one-time bias load"):
            nc.gpsimd.dma_start(bq_s, bq.rearrange("(m p) -> p m", p=P))
            nc.gpsimd.dma_start(bk_s, bk.rearrange("(m p) -> p m", p=P))
        bv_row = consts.tile([P, CW], F32)
        nc.gpsimd.dma_start(bv_row, bv[None, :].to_broadcast([P, CW]))

        # --- HAM warm-up fillers (run while the first DMA chunks land) -----
        warm_ps = psum.tile([P, 2, FN], F32, tag="ps", name="warm_ps")
        for i in range(NWARM):
            nc.tensor.matmul(warm_ps[:, i % 2], ones_s, warm_rhs,
                             start=True, stop=True)

        qt = qkv.tile([P, HG, S], BF16)     # Q^T: [d-part, head, S]
        kt_sb = qkv.tile([P, HG, S], BF16)  # K^T
        v_sb = qkv.tile([P, SK, CW], BF16)  # V: [S-part(tile), S-tile, 4*d]
        wo_s = wo_pool.tile([P, HG, H], BF16)

        # phase-B pools are opened lazily (SBUF is tight during phase A);
        # these names are captured by the closures below
        exp_pool = red = rden_pool = ystage = outt = None

        def emit_scores(c, h):
            """scoresT -> exp (ACT) with an incremental DVE denominator
            chase; returns (exp tile, row-sum partial [128k x 512q])."""
            et = exp_pool.tile([P, SK, FN], BF16, name="et")
            pj = [None] * 8
            aj = [None] * 4
            bj = [None] * 2
            dsum = None
            for j in range(8):
                ps = psum.tile([P, 2, FN], F32, tag="ps", name="ps_s")
                for i in range(2):
                    nc.tensor.matmul(
                        ps[:, i],
                        kt_sb[:, h, (2 * j + i) * P:(2 * j + i + 1) * P],
                        qt[:, h, c * FN:(c + 1) * FN],
                        start=True, stop=True,
                    )
                nc.scalar.activation(
                    et[:, 2 * j:2 * j + 2, :].rearrange("p a q -> p (a q)"),
                    ps.rearrange("p a q -> p (a q)"),
                    mybir.ActivationFunctionType.Exp)
                p_t = red.tile([P, FN], BF16, tag="p", bufs=4, name="p_t")
                nc.vector.tensor_add(p_t, et[:, 2 * j, :], et[:, 2 * j + 1, :])
                pj[j] = p_t
                if j % 2 == 1:
                    a_t = red.tile([P, FN], BF16, tag="a", bufs=2, name="a_t")
                    nc.vector.tensor_add(a_t, pj[j - 1], pj[j])
                    aj[j // 2] = a_t
                if j == 3:
                    b_t = red.tile([P, FN], BF16, tag="b", bufs=2, name="b_t")
                    nc.vector.tensor_add(b_t, aj[0], aj[1])
                    bj[0] = b_t
                if j == 7:
                    b_t = red.tile([P, FN], BF16, tag="b", bufs=2, name="b_t")
                    nc.vector.tensor_add(b_t, aj[2], aj[3])
                    bj[1] = b_t
                    dsum = red.tile([P, FN], BF16, tag="d", bufs=3, name="dsum")
                    nc.vector.tensor_add(dsum, bj[0], bj[1])
            return et, dsum

        def emit_av(c, h, et, dsum):
            ps = psum.tile([P, 2, FN], F32, tag="ps", name="ps_av")
            for k in range(SK):
                nc.tensor.matmul(
                    ps[:, 0],
                    v_sb[:, k, h * P:(h + 1) * P],
                    et[:, k, :],
                    start=(k == 0), stop=(k == SK - 1),
                )
            # denominator broadcast AFTER the AV chain: by the time the 16 AV
            # matmuls finish, the DVE chase has long since produced dsum
            nc.tensor.matmul(ps[:, 1], ones_s, dsum, start=True, stop=True)
            rden = rden_pool.tile([P, FN], F32, name="rden")
            nc.vector.reciprocal_approx_fast(out=rden, in_=ps[:, 1])
            nc.vector.tensor_mul(outt[:, h, c * FN:(c + 1) * FN], ps[:, 0], rden)

        def emit_ctile(m):
            for c2p in range(2):
                ps = psum.tile([P, 2, FN], F32, tag="ps", name="ps_c")
                for half in range(2):
                    c2 = 2 * c2p + half
                    for kh in range(HG):
                        nc.tensor.matmul(
                            ps[:, half],
                            outt[:, kh, m * P:(m + 1) * P],
                            wo_s[:, kh, c2 * FN:(c2 + 1) * FN],
                            start=(kh == 0), stop=(kh == HG - 1),
                        )
                yt = ystage.tile([P, 2 * FN], F32, name="yt")
                nc.vector.tensor_copy(yt, ps.rearrange("p a q -> p (a q)"))
                nc.sync.dma_start(
                    y[m * P:(m + 1) * P, c2p * 2 * FN:(c2p + 1) * 2 * FN], yt)

        # --- phase A: projections ------------------------------------------
        with tc.tile_pool(name="xpool", bufs=1) as xpool:
            xts = xpool.tile([P, KT, S], BF16)
            xt_r = xt.rearrange("(a p) s -> p a s", p=P)

            with tc.tile_pool(name="wkq", bufs=1) as wkq:
                wk_s = wkq.tile([P, KT, CW], BF16)
                wq_s = wkq.tile([P, KT, CW], BF16)
                wk_r = wk.rearrange("(a p) m -> p a m", p=P)
                wq_r = wq.rearrange("(a p) m -> p a m", p=P)
                for k in range(KT):
                    nc.sync.dma_start(wk_s[:, k], wk_r[:, k])
                    nc.sync.dma_start(xts[:, k], xt_r[:, k])
                for k in range(KT):
                    nc.sync.dma_start(wq_s[:, k], wq_r[:, k])

                # A1: K^T heads 0,1 -- contraction-outer DMA chase across all
                # 8 PSUM banks; 8 matmuls fire per arriving (wk, xt) chunk
                part1 = [(m, c) for m in (0, 1) for c in range(SQ)]
                a1 = [psum.tile([P, 2, FN], F32, tag="ps", name="a1") for _ in range(4)]
                for k in range(KT):
                    for gi, (m, c) in enumerate(part1):
                        nc.tensor.matmul(
                            a1[gi // 2][:, gi % 2],
                            wk_s[:, k, m * P:(m + 1) * P],
                            xts[:, k, c * FN:(c + 1) * FN],
                            start=(k == 0), stop=(k == KT - 1),
                        )
                for gi, (m, c) in enumerate(part1):
                    nc.vector.tensor_scalar_add(
                        kt_sb[:, m, c * FN:(c + 1) * FN],
                        a1[gi // 2][:, gi % 2], bk_s[:, m:m + 1])

                # A2: K^T heads 2,3 (dense, inputs resident)
                def proj_pair(w_s, b_s, dst, g0, g1):
                    ps = psum.tile([P, 2, FN], F32, tag="ps", name="ps_a")
                    for half, (m, c) in enumerate((g0, g1)):
                        for k in range(KT):
                            nc.tensor.matmul(
                                ps[:, half],
                                w_s[:, k, m * P:(m + 1) * P],
                                xts[:, k, c * FN:(c + 1) * FN],
                                start=(k == 0), stop=(k == KT - 1),
                            )
                    for half, (m, c) in enumerate((g0, g1)):
                        nc.vector.tensor_scalar_add(
                            dst[:, m, c * FN:(c + 1) * FN],
                            ps[:, half], b_s[:, m:m + 1])

                part2 = [(m, c) for m in (2, 3) for c in range(SQ)]
                for i in range(0, 8, 2):
                    proj_pair(wk_s, bk_s, kt_sb, part2[i], part2[i + 1])

                # A3: Q^T (all heads)
                partq = [(m, c) for m in range(HG) for c in range(SQ)]
                for i in range(0, 16, 2):
                    proj_pair(wq_s, bq_s, qt, partq[i], partq[i + 1])

            with tc.tile_pool(name="wvp", bufs=1) as wvp:
                wv_s = wvp.tile([P, KT, CW], BF16)
                wv_r = wv.rearrange("(a p) m -> p a m", p=P)
                for k in range(KT):
                    nc.sync.dma_start(wv_s[:, k], wv_r[:, k])
                nc.sync.dma_start(wo_s, wo.rearrange("(a p) n -> p a n", p=P))

                # A4: V projection
                def v_pair(m0, m1):
                    ps = psum.tile([P, 2, FN], F32, tag="ps", name="ps_v")
                    for half, m in enumerate((m0, m1)):
                        for k in range(KT):
                            nc.tensor.matmul(
                                ps[:, half],
                                xts[:, k, m * P:(m + 1) * P],
                                wv_s[:, k, :],
                                start=(k == 0), stop=(k == KT - 1),
                            )
                    for half, m in enumerate((m0, m1)):
                        nc.vector.tensor_add(v_sb[:, m, :], ps[:, half], bv_row)

                for m in range(0, 16, 2):
                    v_pair(m, m + 1)

        # --- phases B+C, software-pipelined --------------------------------
        exp_pool = ctx.enter_context(tc.tile_pool(name="expt", bufs=3))
        red = ctx.enter_context(tc.tile_pool(name="red", bufs=1))
        rden_pool = ctx.enter_context(tc.tile_pool(name="rden", bufs=2))
        ystage = ctx.enter_context(tc.tile_pool(name="ystage", bufs=3))
        outt = ctx.enter_context(tc.tile_pool(name="outt", bufs=1)).tile(
            [P, HG, S], BF16)

        # emission order interleaves score units, AV units, and the previous
        # chunk's output-projection tiles so that (a) the ACT exp stream never
        # paces the PE through the 4-deep PSUM rotation and (b) every AV unit
        # starts after its exp tile is complete
        for c in range(SQ):
            e0 = emit_scores(c, 0)
            e1 = emit_scores(c, 1)
            emit_av(c, 0, *e0)
            e2 = emit_scores(c, 2)
            emit_av(c, 1, *e1)
            if c > 0:
                emit_ctile(4 * (c - 1) + 0)
            e3 = emit_scores(c, 3)
            if c > 0:
                emit_ctile(4 * (c - 1) + 1)
            emit_av(c, 2, *e2)
            if c > 0:
                emit_ctile(4 * (c - 1) + 2)
            emit_av(c, 3, *e3)
            if c > 0:
                emit_ctile(4 * (c - 1) + 3)
        for m in range(4 * (SQ - 1), 4 * SQ):
            emit_ctile(m)


def _get_nc():
    if "nc" not in _CACHE:
        _CACHE["nc"] = _build_nc()
    return _CACHE["nc"]


def make_in_maps(X, Wq, bq, Wk, bk, Wv, bv, Wo, bo):
    bf16 = ml_dtypes.bfloat16
    scale = 1.0 / math.sqrt(D)
    X = np.asarray(X, dtype=np.float32)
    xt_b = [np.ascontiguousarray(X[b].T).astype(bf16) for b in range(B)]
    Wq = np.asarray(Wq, dtype=np.float32) * scale
    Wk = np.asarray(Wk, dtype=np.float32)
    Wv = np.asarray(Wv, dtype=np.float32)
    Wo = np.asarray(Wo, dtype=np.float32)
    bq = np.asarray(bq, dtype=np.float32) * scale
    bk = np.asarray(bk, dtype=np.float32)
    bv = np.asarray(bv, dtype=np.float32)
    in_maps = []
    for c in range(NC):
        b, g = divmod(c, GROUPS)
        sl = slice(g * CW, (g + 1) * CW)
        in_maps.append({
            "xt": xt_b[b],
            "wq": np.ascontiguousarray(Wq[:, sl]).astype(bf16),
            "wk": np.ascontiguousarray(Wk[:, sl]).astype(bf16),
            "wv": np.ascontiguousarray(Wv[:, sl]).astype(bf16),
            "wo": np.ascontiguousarray(Wo[sl, :]).astype(bf16),
            "bq": np.ascontiguousarray(bq[sl]),
            "bk": np.ascontiguousarray(bk[sl]),
            "bv": np.ascontiguousarray(bv[sl]),
        })
    return in_maps


def gather_output(results, bo):
    bo = np.asarray(bo, dtype=np.float32)
    out = np.empty((B, S, H), np.float32)
    for b in range(B):
        acc = results[b * GROUPS]["y"].astype(np.float32, copy=True)
        for g in range(1, GROUPS):
            acc += results[b * GROUPS + g]["y"]
        out[b] = acc + bo[None, :]
    return out


def kernel(X, Wq, bq, Wk, bk, Wv, bv, Wo, bo):
    from concourse.bass_utils import run_bass_kernel_spmd

    in_maps = make_in_maps(X, Wq, bq, Wk, bk, Wv, bv, Wo, bo)
    nc = _get_nc()
    res = run_bass_kernel_spmd(nc, in_maps, list(range(NC))).results
    return gather_output(res, bo)
